# revision 9
# baseline (speedup 1.0000x reference)
"""Multi-head attention (no qkv proj) + out_proj, sharded over 8 TRN2 cores.

Sharding: core i handles batch b = i//4, query rows tc = (i//2)%2 of 512,
and head group hg = i%2 (8 of 16 heads).  out_proj weight is column-sharded
over the head groups; the all-reduce is a host-side partial-sum of the two
head-group outputs at gather time.

v2 pipeline (exp on the scalar engine is the pacing engine at ~32us):
  - heads processed in PAIRS (2h, 2h+1).  Score matmuls for the two heads
    of a pair are row-tiled (K=64 each, tile_position (0,0)/(64,0)) so they
    run CONCURRENTLY on the PE array, writing the two banks of one
    [128, 1024] PSUM tile.
  - one ACT instruction exps the whole [128, 1024] pair-chunk from PSUM.
  - exp(bias) (host-precomputed fp16, mask rows = 0) multiplies in on DVE.
  - AV per head: [V_h | 1]^T @ expv accumulated over 8 s-chunks (65th row
    = softmax denominator).
  - normalize: DVE fast reciprocal straight from PSUM, fp32 K=1 matmul
    broadcasts 1/den across 64 partitions, DVE copy + multiply.
  - out_proj: wT-chunks^T @ attnflatT + out_b at the end.
  - no big-N warmup matmuls: ACT paces the middle, so a cold PE never
    stalls the pipeline; tiny keeper matmuls prevent HAM re-throttle gaps.
  - DMA: few large transfers with host-packed layouts (contiguous 1-16KB
    per-partition lines), issued earliest-needed-first.
"""

import numpy as np

import concourse.mybir as mybir
import concourse.tile as tile
from concourse import bacc
from concourse.bass_utils import run_bass_kernel_spmd

F32 = mybir.dt.float32
F16 = mybir.dt.float16
NP16 = np.float16

P = 128          # partitions
T = 512          # query rows per core
S = 1024         # key length
H = 8            # heads per core (of 16)
NPAIR = H // 2   # head pairs
HD = 64          # head dim
DIN = H * HD     # local d_model slice (512)
NDIN = DIN // P  # 4 chunks
DM = 1024        # full d_model
NS = S // P      # 8 s-chunks
ND = DM // P     # 8 d_out chunks
SCALE = HD ** -0.5
EXP_SHIFT = -2.0  # exp(x-2): keeps fp16 exp outputs well inside range

AF = mybir.ActivationFunctionType
ALU = mybir.AluOpType


def build_bass():
    nc = bacc.Bacc()

    qT_d = nc.dram_tensor("qT", [P, NDIN * T], F16, kind="ExternalInput")
    kT_d = nc.dram_tensor("kT", [P, NDIN * S], F16, kind="ExternalInput")
    vaug_d = nc.dram_tensor("vaug", [P, NS * H * (HD + 1)], F16, kind="ExternalInput")
    # per head-pair: [p, sc, head(2), t] fp16, exp(bias) with mask rows = 0
    bias_d = nc.dram_tensor("biasT", [NPAIR, P, NS * 2 * T], F16, kind="ExternalInput")
    wT_d = nc.dram_tensor("wT", [P, NDIN * DM], F16, kind="ExternalInput")
    outb_d = nc.dram_tensor("outb", [P, ND], F32, kind="ExternalInput")
    outT_d = nc.dram_tensor("outT", [DM, T], F16, kind="ExternalOutput")

    with tile.TileContext(nc) as tc, nc.allow_low_precision(reason="fp16 matmul pipeline"):
        with (
            tc.tile_pool(name="weights", bufs=1) as wpool,
            tc.tile_pool(name="bias", bufs=3) as bpool,
            tc.tile_pool(name="rows", bufs=2) as rpool,
            tc.tile_pool(name="small", bufs=2) as spool,
            tc.tile_pool(name="osb", bufs=1) as opool_sb,
        ):
            qT_t = [wpool.tile([P, T], F16, name=f"qT{c}", tag=f"qT{c}")
                    for c in range(NDIN)]
            kT_t = [wpool.tile([P, S], F16, name=f"kT{c}", tag=f"kT{c}")
                    for c in range(NDIN)]
            vaug_t = wpool.tile([P, NS * H * (HD + 1)], F16, name="va", tag="va")
            wT_t = wpool.tile([P, NDIN * DM], F16, name="wT", tag="wT")
            outb_t = wpool.tile([P, ND], F32, name="outb", tag="outb")
            ones16_t = wpool.tile([P, HD], F16, name="ones", tag="ones")
            eshift_t = wpool.tile([P, 1], F32, name="eshift", tag="eshift")
            warm_t = wpool.tile([P, P], F16, name="warm", tag="warm")
            aflat_t = [wpool.tile([P, T], F16, name=f"af{c}", tag=f"af{c}")
                       for c in range(NDIN)]

            # memsets on gpsimd: no dependency on the vector engine's startup
            nc.gpsimd.memset(ones16_t[:], 1.0)
            nc.gpsimd.memset(eshift_t[:], EXP_SHIFT)
            nc.gpsimd.memset(warm_t[:], 0.0)

            # ---- input DMAs, earliest-needed-first --------------------
            nc.sync.dma_start(out=qT_t[0][:], in_=qT_d[:, 0:T])
            nc.sync.dma_start(out=kT_t[0][:], in_=kT_d[:, 0:S])

            with (
                tc.tile_pool(name="scps", bufs=2, space="PSUM") as scps,
                tc.tile_pool(name="avps", bufs=2, space="PSUM") as avps,
                tc.tile_pool(name="bcps", bufs=1, space="PSUM") as bcps,
                tc.tile_pool(name="warmps", bufs=1, space="PSUM") as warmps,
            ):
                wm_ps = warmps.tile([P, HD], F32, name="wm", tag="wm")

                bias_t = [None] * NPAIR
                bias_t[0] = bpool.tile([P, NS * 2 * T], F16, name="eb", tag="eb")
                nc.sync.dma_start(out=bias_t[0][:], in_=bias_d[0, :, :])
                nc.sync.dma_start(out=vaug_t[:], in_=vaug_d[:, :])
                for c in range(1, NDIN):
                    nc.sync.dma_start(out=kT_t[c][:], in_=kT_d[:, c * S:(c + 1) * S])
                    nc.sync.dma_start(out=qT_t[c][:], in_=qT_d[:, c * T:(c + 1) * T])
                bias_t[1] = bpool.tile([P, NS * 2 * T], F16, name="eb", tag="eb")
                nc.sync.dma_start(out=bias_t[1][:], in_=bias_d[1, :, :])
                nc.sync.dma_start(out=outb_t[:], in_=outb_d[:, :])

                for pr in range(NPAIR):
                    ha = slice(0, HD)          # head a: d-rows 0:64 of chunk pr
                    hb = slice(HD, P)          # head b: d-rows 64:128

                    # prefetch next pair's exp(bias)
                    if pr + 2 < NPAIR:
                        bias_t[pr + 2] = bpool.tile([P, NS * 2 * T], F16,
                                                    name="eb", tag="eb")
                        nc.sync.dma_start(out=bias_t[pr + 2][:],
                                          in_=bias_d[pr + 2, :, :])
                    if pr == 2:
                        nc.sync.dma_start(out=wT_t[:], in_=wT_d[:, :])

                    expv = rpool.tile([P, NS * 2 * T], F16, name="expv", tag="expv")
                    for sc in range(NS):
                        sc_ps = scps.tile([P, 2 * T], F32, name="sc", tag="sc")
                        ksl = slice(sc * P, (sc + 1) * P)
                        # two row-tiled matmuls run concurrently (K=64 each),
                        # filling the two PSUM banks of this tile
                        nc.tensor.matmul(
                            sc_ps[:, 0:T],
                            kT_t[pr][ha, ksl], qT_t[pr][ha, :],
                            start=True, stop=True,
                        )
                        nc.tensor.matmul(
                            sc_ps[:, T:2 * T],
                            kT_t[pr][hb, ksl], qT_t[pr][hb, :],
                            start=True, stop=True,
                        )
                        esl = slice(sc * 2 * T, (sc + 1) * 2 * T)
                        nc.scalar.activation(
                            expv[:, esl], sc_ps[:], AF.Exp,
                            bias=eshift_t[:], scale=SCALE,
                        )
                        if sc % 2 == 1:
                            msl = slice((sc - 1) * 2 * T, (sc + 1) * 2 * T)
                            nc.vector.tensor_mul(
                                expv[:, msl], expv[:, msl], bias_t[pr][:, msl])

                    for hi, hsl in ((0, ha), (1, hb)):
                        h = 2 * pr + hi
                        av_ps = avps.tile([HD + 1, T], F32, name="av", tag="av")
                        for sc in range(NS):
                            nc.tensor.matmul(
                                av_ps[:],
                                vaug_t[:, (sc * H + h) * (HD + 1):
                                          (sc * H + h + 1) * (HD + 1)],
                                expv[:, sc * 2 * T + hi * T: sc * 2 * T + (hi + 1) * T],
                                start=(sc == 0), stop=(sc == NS - 1),
                            )
                        # keeper matmul: prevents a HAM MID-window re-throttle
                        nc.tensor.matmul(wm_ps[:], warm_t[:], warm_t[:, 0:HD],
                                         start=True, stop=True)
                        den_sb = spool.tile([1, T], F32, name="den_sb", tag="den_sb")
                        nc.vector.tensor_copy(den_sb[:], av_ps[HD:HD + 1, :])
                        rcp = spool.tile([1, T], F32, name="rcp", tag="rcp")
                        nc.vector.reciprocal_approx_fast(rcp[:], den_sb[:])
                        rcp16 = spool.tile([1, T], F16, name="rcp16", tag="rcp16")
                        nc.vector.tensor_copy(rcp16[:], rcp[:])
                        bc_ps = bcps.tile([HD, T], F32, name="bcp", tag="bcp")
                        nc.tensor.matmul(
                            bc_ps[:], ones16_t[0:1, :], rcp16[:],
                            start=True, stop=True,
                        )
                        bc_sb = spool.tile([HD, T], F32, name="bc", tag="bc", bufs=4)
                        nc.scalar.copy(bc_sb[:], bc_ps[:])
                        nc.vector.tensor_mul(
                            aflat_t[pr][hsl, :], av_ps[0:HD, :], bc_sb[:],
                        )

            # ---- out_proj: outT[dout, t] = W^T-slice @ attnflatT (+ out_b) ----
            osb = opool_sb.tile([P, ND * T], F16, name="osb", tag="osb")
            with tc.tile_pool(name="ops", bufs=4, space="PSUM") as ops:
                for dc in range(ND):
                    o_ps = ops.tile([P, T], F32, name="o", tag="o")
                    for dinc in range(NDIN):
                        nc.tensor.matmul(
                            o_ps[:],
                            wT_t[:, dinc * DM + dc * P: dinc * DM + (dc + 1) * P],
                            aflat_t[dinc][:],
                            start=(dinc == 0), stop=(dinc == NDIN - 1),
                        )
                    osl = slice(dc * T, (dc + 1) * T)
                    if dc % 2 == 0:
                        nc.scalar.activation(
                            osb[:, osl], o_ps[:], AF.Identity,
                            bias=outb_t[:, dc:dc + 1],
                        )
                    else:
                        nc.vector.tensor_scalar_add(
                            osb[:, osl], o_ps[:], outb_t[:, dc:dc + 1],
                        )
                    nc.sync.dma_start(
                        out=outT_d[dc * P:(dc + 1) * P, :],
                        in_=osb[:, osl],
                    )

    nc.finalize()
    return nc


_NC = None


def _get_nc():
    global _NC
    if _NC is None:
        _NC = build_bass()
    return _NC


def _core_index(b, tc_i, hg):
    return b * 4 + tc_i * 2 + hg


def _pack_chunks(arr2d, nchunk):
    """[nchunk*128, F] -> [128, nchunk*F] with chunk-major free dim."""
    f = arr2d.shape[1]
    return np.ascontiguousarray(
        arr2d.reshape(nchunk, P, f).transpose(1, 0, 2).reshape(P, nchunk * f))


def _make_in_maps(query, key, value, attn_bias, key_padding_mask, out_w, out_b):
    query = np.asarray(query, dtype=np.float32)
    key = np.asarray(key, dtype=np.float32)
    value = np.asarray(value, dtype=np.float32)
    attn_bias = np.asarray(attn_bias, dtype=np.float32)
    mask = np.asarray(key_padding_mask).astype(bool)
    out_w = np.asarray(out_w, dtype=np.float32)
    out_b = np.asarray(out_b, dtype=np.float32)

    wT_full = np.ascontiguousarray(out_w.T).astype(NP16)   # [din, dout]
    outb = np.ascontiguousarray(out_b.reshape(ND, P).T)
    outb0 = np.zeros_like(outb)

    maps = [None] * 8
    for b in range(2):
        kT_full = np.ascontiguousarray(key[b].T).astype(NP16)  # [1024, 1024]
        for hg in range(2):
            hs = hg * H              # first global head of the group
            ds = hg * DIN            # first d_model row of the group
            vaug = np.ones((S, H, HD + 1), NP16)
            vaug[:, :, :HD] = value[b, :, ds:ds + DIN].reshape(S, H, HD)
            vaug_p = _pack_chunks(vaug.reshape(S, H * (HD + 1)), NS)
            kT = _pack_chunks(kT_full[ds:ds + DIN], NDIN)
            wT = _pack_chunks(wT_full[ds:ds + DIN], NDIN)
            for tc_i in range(2):
                t0 = tc_i * T
                qT = _pack_chunks(np.ascontiguousarray(
                    query[b, t0:t0 + T, ds:ds + DIN].T).astype(NP16), NDIN)
                ebias = np.ascontiguousarray(
                    attn_bias[b, hs:hs + H, t0:t0 + T, :].transpose(0, 2, 1))
                ebias[:, mask[b], :] = -10000.0
                np.exp(ebias, out=ebias)                       # [H, S, T]
                # [pair, p, sc*2*T]: (pair, sc, head-in-pair, t)
                ebp = np.ascontiguousarray(
                    ebias.astype(NP16)
                    .reshape(NPAIR, 2, NS, P, T)               # pr, hd, sc, p, t
                    .transpose(0, 3, 2, 1, 4)                  # pr, p, sc, hd, t
                    .reshape(NPAIR, P, NS * 2 * T))
                maps[_core_index(b, tc_i, hg)] = {
                    "qT": qT, "kT": kT, "vaug": vaug_p,
                    "biasT": ebp,
                    "wT": wT, "outb": outb if hg == 0 else outb0,
                }
    return maps


def run(inputs, trace=False, **run_kwargs):
    """Returns (output [2,1024,1024] f32, BassKernelResults)."""
    nc = _get_nc()
    in_maps = _make_in_maps(**inputs)
    res = run_bass_kernel_spmd(
        nc, in_maps, core_ids=list(range(8)), trace=trace, **run_kwargs
    )
    out = np.empty((2, S, DM), np.float32)
    for b in range(2):
        for tc_i in range(2):
            part = (np.asarray(res.results[_core_index(b, tc_i, 0)]["outT"], dtype=np.float32)
                    + np.asarray(res.results[_core_index(b, tc_i, 1)]["outT"], dtype=np.float32))
            out[b, tc_i * T:(tc_i + 1) * T, :] = part.T
    return out, res


def kernel(**inputs):
    out, _ = run(inputs, trace=False)
    return out
